# revision 63
# baseline (speedup 1.0000x reference)
"""Multi-head attention with fraction-based RoPE ("stoich RoPE") on 8
Trainium2 NeuronCores.

Sharding: each core owns one (batch, query-half) pair — B=4 batches x 2
query halves = 8 shards.  Every core projects Q for its 1024 query rows
and K/V for the full 2048 keys of its batch (K/V projection is computed
on both cores sharing a batch; the 2x redundancy buys a kernel with no
collectives: the attention output rows owned by a core carry the full
head dimension, so the output projection and bias are entirely local).

Per-core device program (SPMD, identical on all 8 cores):
  phase A  per head-pair (8 x 128 dims): project Q^T/K^T/V^T from x^T
           streamed out of DRAM (weights stationary, x moving), add
           biases, apply RoPE to Q/K via precomputed cos/sin tiles and
           32-partition cross-quadrant swaps, PE-transpose V into
           natural layout with a ones column appended (row 64 of the
           P@V' output then carries the softmax denominator).
  phase B  attention per head: scores^T = K^T.T @ Q^T chunks -> exp on
           ACT (scale=1/8 folded in, no max subtraction: |scores/8| is
           O(1) for this operator's input distribution) -> P^T@V'
           accumulation -> reciprocal + K=1 broadcast matmul ->
           normalized attn^T written per pair region.
  phase C  output projection: attn^T chunks stationary, Wo^T moving,
           + bias, DMA out rows.

The host shards/formats inputs (transposes, bias/cos-sin tiles) and
concatenates the 8 output row-shards.
"""

import contextlib
import ctypes
import sys
import types

import numpy as np
import ml_dtypes

import concourse.bass as bass
import concourse.mybir as mybir
import concourse.tile as tile
from concourse import library_config
from concourse.masks import make_identity
from concourse.vector_clock import ScopedClock

# ---------------- problem constants (hardcoded per contract) ----------------
B, T, D = 4, 2048, 1024
H, HD = 16, 64  # heads, head dim
HALF = HD // 2
N_CORES = 8
TQ = T // 2  # query rows per core
P = 128
NQ = 512  # moving-dim tile for matmuls
NPAIR = D // P  # 8 head pairs per core
SCALE = 1.0 / np.sqrt(HD)  # folded into exp()
ROPE_SCALE = 1000.0
ROPE_BASE = 10000.0

F32 = mybir.dt.float32
DT_MM = mybir.dt.bfloat16  # dtype of matmul operands (bfloat16 | float32)

_SO_PATH = "/opt/axon/libaxon_pjrt.so"


# ---------------- axon/NTFF environment shims ----------------
def _ntff_profile_hook():
    try:
        lib = ctypes.CDLL(_SO_PATH)
    except OSError:
        return None
    if not hasattr(lib, "axon_start_nrt_profile"):
        return None
    lib.axon_start_nrt_profile.argtypes = [
        ctypes.POINTER(ctypes.c_int64),
        ctypes.c_size_t,
    ]
    lib.axon_start_nrt_profile.restype = ctypes.c_int64
    lib.axon_stop_nrt_profile.argtypes = [ctypes.c_char_p]
    lib.axon_stop_nrt_profile.restype = ctypes.c_int64

    @contextlib.contextmanager
    def _hook(output_dir, device_ids):
        import jax

        jax.devices()
        if device_ids:
            ids = (ctypes.c_int64 * len(device_ids))(*device_ids)
            rc = lib.axon_start_nrt_profile(ids, len(device_ids))
        else:
            rc = lib.axon_start_nrt_profile(None, 0)
        if rc != 0:
            raise RuntimeError(f"axon_start_nrt_profile rc={rc}")
        try:
            yield
        finally:
            n = lib.axon_stop_nrt_profile(str(output_dir).encode())
            if n < 0:
                raise RuntimeError(f"axon_stop_nrt_profile rc={n}")

    return _hook


def install_shims():
    if "antenv.axon_hooks" not in sys.modules:
        mod = types.ModuleType("antenv.axon_hooks")
        hook = _ntff_profile_hook()
        mod.get_axon_ntff_profile_hook = lambda: hook
        mod.set_axon_ntff_profile_hook = lambda h: None
        sys.modules["antenv.axon_hooks"] = mod
    import concourse.bass_utils as bass_utils

    bass_utils.upload_artifacts = lambda tmpdir: str(tmpdir)

    import os

    if os.environ.get("BASS_LDW_OPT") == "1" and not getattr(
        bass_utils, "_ldw_opt_patched", False
    ):
        orig_run = bass_utils.run_command

        def _run_ldw(argv, **kw):
            argv = [
                "--enable-ldw-opt=true" if a == "--enable-ldw-opt=false" else a
                for a in argv
            ]
            return orig_run(argv, **kw)

        bass_utils.run_command = _run_ldw
        bass_utils._ldw_opt_patched = True


class TileContextSplitDrain(tile.TileContext):
    """This walrus build encodes at most 2 sync waits per CTRL
    instruction; Tile's kernel-tail drain wants one wait per logical
    processor.  Split the waits across single-wait NOPs instead."""

    MAX_WAITS = 1

    def _drain_and_barrier(self, tick_clock, wait_clock):
        nc = self.nc
        carrier = nc.sync.nop(nofuse=True)
        wait_clock.add_sem_waits(
            carrier.ins, ScopedClock({None: tick_clock.global_clock})
        )
        waits = list(carrier.ins.sync_info.on_wait or [])
        if len(waits) > self.MAX_WAITS:
            carrier.ins.sync_info.on_wait[:] = waits[: self.MAX_WAITS]
            for i in range(self.MAX_WAITS, len(waits), self.MAX_WAITS):
                extra = nc.sync.nop(nofuse=True)
                extra.ins.sync_info = mybir.SyncInfo(
                    on_wait=list(waits[i : i + self.MAX_WAITS]), on_update=[]
                )
        nc.sync.drain()
        nc.all_engine_barrier()
        assert self.sems is not None
        popped = nc._tile_sem_poison_stack.pop()
        assert popped is self._sem_poison
        nc.clear_and_free_semaphores(list(self.sems.allocated().values()))
        nc.all_engine_barrier()


def _split_sync_waits(nc, max_waits=1):
    """This walrus build rejects instructions carrying more than a couple
    of sync waits (matmul takes 2, activation only 1).  Move excess waits
    onto same-engine NOPs inserted just before the instruction (AND
    semantics are preserved: the engine blocks on each carrier in program
    order)."""
    for f in nc.m.functions:
        for bb in f.blocks:
            out = []
            for inst in bb.instructions:
                mw = max_waits
                si = inst.sync_info
                waits = list(si.on_wait) if si and si.on_wait else []
                if len(waits) > mw:
                    for i in range(0, len(waits) - mw, mw):
                        nop = mybir.InstNoOp(
                            name=nc.get_next_instruction_name(), ins=[], outs=[]
                        )
                        nop.engine = inst.engine
                        nop.sync_info = mybir.SyncInfo(
                            on_wait=list(waits[i : i + mw]), on_update=[]
                        )
                        nc.register_instruction(nop, overwrite=True)
                        out.append(nop)
                    si.on_wait[:] = waits[len(waits) - mw :]
                out.append(inst)
            bb.instructions[:] = out


# ---------------- device program ----------------
def build_nc(dt_mm=DT_MM):
    nc = bass.Bass(
        "TRN2", target_bir_lowering=False, debug=False, num_devices=N_CORES
    )

    # x/weight layouts are host-blocked so each device DMA is contiguous
    # per partition (128 descriptors instead of 1024 per transfer):
    #   xtb[nb, p, f, t'] = x.T[f*128+p, nb*NQ+t']   (same for xtqb)
    #   wXb[pr, p, f, d'] = WX.T[f*128+p, pr*128+d']
    xtb = nc.dram_tensor("xtb", [T // NQ, P, NPAIR, NQ], dt_mm, kind="ExternalInput")
    xtqb = nc.dram_tensor(
        "xtqb", [TQ // NQ, P, NPAIR, NQ], dt_mm, kind="ExternalInput"
    )
    wqtb = nc.dram_tensor("wqtb", [NPAIR, P, NPAIR, P], dt_mm, kind="ExternalInput")
    wktb = nc.dram_tensor("wktb", [NPAIR, P, NPAIR, P], dt_mm, kind="ExternalInput")
    wvtb = nc.dram_tensor("wvtb", [NPAIR, P, NPAIR, P], dt_mm, kind="ExternalInput")
    wot = nc.dram_tensor("wot", [D, D], dt_mm, kind="ExternalInput")
    bq = nc.dram_tensor("bq", [P, NPAIR], F32, kind="ExternalInput")
    bk = nc.dram_tensor("bk", [P, NPAIR], F32, kind="ExternalInput")
    bv = nc.dram_tensor("bv", [P, NPAIR], F32, kind="ExternalInput")
    bob = nc.dram_tensor("bob", [P, D], F32, kind="ExternalInput")
    # cos/sin tiles are 4x row-replications of a [32, T] pattern; the host
    # sends 32 rows, gpsimd replicates on-chip (saves 1.1MB of startup DMA)
    csaq = nc.dram_tensor("csaq", [HALF, TQ], dt_mm, kind="ExternalInput")
    csbq = nc.dram_tensor("csbq", [HALF, TQ], dt_mm, kind="ExternalInput")
    csak = nc.dram_tensor("csak", [HALF, T], dt_mm, kind="ExternalInput")
    csbk = nc.dram_tensor("csbk", [HALF, T], dt_mm, kind="ExternalInput")
    out = nc.dram_tensor("out", [TQ, D], F32, kind="ExternalOutput")
    # DRAM bounce buffers: softmax denominators go out as a [1, 4NQ] row
    # and come back as [128, 16] so the reciprocal runs on 128 lanes.
    sumsd = nc.dram_tensor("sumsd", [NPAIR, 4 * NQ], F32, kind="Internal")
    recd = nc.dram_tensor("recd", [NPAIR, 4 * NQ], dt_mm, kind="Internal")
    # V staging for the XBAR transpose.  Row layout = the PV lhsT column
    # layout: rows 0:64 head-0 dims, row 64 ones, 65:128 zeros (-> vn_h0);
    # rows 128 ones, 129:192 zeros, 192:256 head-1 dims (-> vn_h1).  The
    # ones/zero rows are written once at startup; per pair only the dim
    # rows are overwritten (pair pr+1's write naturally waits for pair
    # pr's transposes, which complete early in pr's attention).
    vtd = nc.dram_tensor("vtd", [2 * P, T], dt_mm, kind="Internal")

    with TileContextSplitDrain(nc) as tc:
        persist_cm = tc.tile_pool(name="persist", bufs=1)
        persist = persist_cm.__enter__()

        def ptile(shape, dt, tag):
            return persist.tile(shape, dt, tag=tag, name=tag)

        # pools that outlive the attention scope (pair 7's normalization
        # units are pumped inside the output-projection scope)
        outer_cm = contextlib.ExitStack()
        aup = outer_cm.enter_context(tc.tile_pool(name="aup", bufs=2))
        recp = outer_cm.enter_context(tc.tile_pool(name="recp", bufs=2))
        pbpool = {}

        with contextlib.ExitStack() as ctx:
            # ---- persistent tiles ----
            csaq_t = ptile([P, TQ], dt_mm, "csaq_t")
            csbq_t = ptile([P, TQ], dt_mm, "csbq_t")
            csak_t = ptile([P, T], dt_mm, "csak_t")
            csbk_t = ptile([P, T], dt_mm, "csbk_t")
            bq_t = ptile([P, NPAIR], F32, "bq_t")
            bk_t = ptile([P, NPAIR], F32, "bk_t")
            bv_t = ptile([P, NPAIR], F32, "bv_t")
            attn = [ptile([P, TQ], dt_mm, f"attn{pr}") for pr in range(NPAIR)]
            ones64 = ptile([1, HD], dt_mm, "ones64")
            nc.vector.memset(ones64[:], 1.0)

            # biases and the 32-row cos/sin patterns are tiny and gate the
            # first PSUM evictions / ropes: load them before everything
            nc.sync.dma_start(bq_t[:], bq[:])
            nc.sync.dma_start(bk_t[:], bk[:])
            nc.sync.dma_start(bv_t[:], bv[:])
            nc.sync.dma_start(csaq_t[0:HALF, :], csaq[:])
            nc.sync.dma_start(csbq_t[0:HALF, :], csbq[:])
            nc.sync.dma_start(csak_t[0:HALF, :], csak[:])
            nc.sync.dma_start(csbk_t[0:HALF, :], csbk[:])
            # replicate to 128 rows on DVE (idle at startup; gpsimd is far
            # too slow): csa = 4x cos pattern; csb = [sin; -sin; sin; -sin]
            for cs_t in (csaq_t, csak_t):
                for r in (1, 2, 3):
                    nc.vector.tensor_copy(
                        cs_t[r * HALF : (r + 1) * HALF, :], cs_t[0:HALF, :]
                    )
            for cs_t in (csbq_t, csbk_t):
                nc.vector.tensor_copy(cs_t[2 * HALF : 3 * HALF, :], cs_t[0:HALF, :])
                for r in (1, 3):
                    nc.vector.tensor_scalar_mul(
                        cs_t[r * HALF : (r + 1) * HALF, :], cs_t[0:HALF, :], -1.0
                    )

            # ---- pools for the head-pair loop ----
            big = 2 if dt_mm != F32 else 1
            xp = ctx.enter_context(tc.tile_pool(name="xp", bufs=3))
            wp = ctx.enter_context(tc.tile_pool(name="wp", bufs=2))
            rawp = ctx.enter_context(tc.tile_pool(name="rawp", bufs=2))
            ropep = ctx.enter_context(tc.tile_pool(name="ropep", bufs=1))
            vtp = ctx.enter_context(tc.tile_pool(name="vtp", bufs=1))
            qkp = ctx.enter_context(tc.tile_pool(name="qkp", bufs=big))
            vnp = ctx.enter_context(tc.tile_pool(name="vnp", bufs=big))
            exp_p = ctx.enter_context(tc.tile_pool(name="exp_p", bufs=6))
            sumsp = ctx.enter_context(tc.tile_pool(name="sumsp", bufs=2))
            ztp = ctx.enter_context(tc.tile_pool(name="ztp", bufs=1))
            ps_proj = ctx.enter_context(
                tc.tile_pool(name="ps_proj", bufs=2, space="PSUM")
            )
            ps_sc = ctx.enter_context(
                tc.tile_pool(name="ps_sc", bufs=2, space="PSUM")
            )
            ps_po = ctx.enter_context(
                tc.tile_pool(name="ps_po", bufs=2, space="PSUM")
            )
            pbpool["pool"] = ps_sc

            def rope_chunk(raw, csa_t, csb_t, out_tile, c0, c1):
                # one [P, c1-c0] chunk: out = raw*csa + swap32(raw*csb),
                # the swap done by partition-shifted multiplies
                n = c1 - c0
                cs = slice(c0, c1)
                m1 = ropep.tile([P, NQ], dt_mm, tag="m1", name="m1")
                m2s = ropep.tile([P, NQ], dt_mm, tag="m2s", name="m2s")
                nc.vector.tensor_mul(m1[:, :n], raw[:, cs], csa_t[:, cs])
                for blk in range(2):
                    b0 = blk * 64
                    nc.vector.tensor_mul(
                        m2s[b0 : b0 + 32, :n],
                        raw[b0 + 32 : b0 + 64, cs],
                        csb_t[b0 + 32 : b0 + 64, cs],
                    )
                    nc.vector.tensor_mul(
                        m2s[b0 + 32 : b0 + 64, :n],
                        raw[b0 : b0 + 32, cs],
                        csb_t[b0 : b0 + 32, cs],
                    )
                nc.vector.tensor_add(out_tile[:, cs], m1[:, :n], m2s[:, :n])

            def stage_units(pr):
                """Emission units for pair pr's projections + RoPE + V
                transpose.  Each unit emits a small instruction group; the
                attention loop of the previous pair pumps these so the PE
                stays dense while ACT works on exp."""
                d0 = pr * P
                st = {}
                units = []

                def u_wdma():
                    st["wq"] = wp.tile([P, NPAIR, P], dt_mm, tag="wq", name="wq_c")
                    st["wk"] = wp.tile([P, NPAIR, P], dt_mm, tag="wk", name="wk_c")
                    st["wv"] = wp.tile([P, NPAIR, P], dt_mm, tag="wv", name="wv_c")
                    # K first: the projection matmul stream starts with wk
                    nc.sync.dma_start(st["wk"][:], wktb[pr])
                    st["qraw"] = rawp.tile([P, TQ], dt_mm, tag="qraw", name="q_raw")
                    st["kraw"] = rawp.tile([P, T], dt_mm, tag="kraw", name="k_raw")
                    st["vt"] = vtp.tile([P, T], dt_mm, tag="vt", name="v_t")

                def u_wdma2():
                    nc.sync.dma_start(st["wv"][:], wvtb[pr])
                    nc.sync.dma_start(st["wq"][:], wqtb[pr])

                units.append(u_wdma)

                def u_xdma(key, nb, src):
                    def go():
                        xc = xp.tile([P, NPAIR, NQ], dt_mm, tag="xc", name="xc")
                        nc.sync.dma_start(xc[:], src[nb])
                        st[key] = xc

                    return go

                def u_mm(w_key, x_key, f, start, stop):
                    def go():
                        if start:
                            st["ps"] = ps_proj.tile([P, NQ], F32, tag="ps", name="ps")
                        nc.tensor.matmul(
                            st["ps"][:],
                            st[w_key][:, f, :],
                            st[x_key][:, f, :],
                            start=start,
                            stop=stop,
                        )

                    return go

                def u_evict(b_t, dst_key, dslice):
                    def go():
                        nc.scalar.activation(
                            st[dst_key][:, dslice],
                            st["ps"][:],
                            mybir.ActivationFunctionType.Identity,
                            bias=b_t[:, pr : pr + 1],
                        )

                    return go

                # all DMAs first: deep prefetch so pumped matmuls never
                # wait on HBM
                units.append(u_xdma("x0", 0, xtb))
                units.append(u_wdma2)
                for nb in range(1, T // NQ):
                    units.append(u_xdma("x%d" % nb, nb, xtb))
                for nb in range(TQ // NQ):
                    units.append(u_xdma("q%d" % nb, nb, xtqb))
                def u_rope_k(nb):
                    def go():
                        if nb == 0:
                            st["kt"] = qkp.tile([P, T], dt_mm, tag="kt", name="kt")
                        rope_chunk(
                            st["kraw"], csak_t, csbk_t, st["kt"],
                            nb * NQ, (nb + 1) * NQ,
                        )

                    return go

                def u_rope_q(nb):
                    def go():
                        if nb == 0:
                            st["qt"] = qkp.tile([P, TQ], dt_mm, tag="qt", name="qt")
                        rope_chunk(
                            st["qraw"], csaq_t, csbq_t, st["qt"],
                            nb * NQ, (nb + 1) * NQ,
                        )

                    return go

                for nb in range(T // NQ):
                    for w_key, b_t, dst_key in (("wk", bk_t, "kraw"), ("wv", bv_t, "vt")):
                        for f in range(NPAIR):
                            units.append(
                                u_mm(w_key, "x%d" % nb, f, f == 0, f == NPAIR - 1)
                            )
                        units.append(
                            u_evict(b_t, dst_key, slice(nb * NQ, (nb + 1) * NQ))
                        )
                    units.append(u_rope_k(nb))
                for nb in range(TQ // NQ):
                    for f in range(NPAIR):
                        units.append(u_mm("wq", "q%d" % nb, f, f == 0, f == NPAIR - 1))
                    units.append(
                        u_evict(bq_t, "qraw", slice(nb * NQ, (nb + 1) * NQ))
                    )
                    units.append(u_rope_q(nb))

                def u_vtd():
                    # stage V^T dims into the DRAM transpose buffer; the
                    # constant ones/zero rows are already there
                    nc.sync.dma_start(vtd[0:HD, :], st["vt"][0:HD, :])
                    nc.sync.dma_start(vtd[3 * HD : 4 * HD, :], st["vt"][HD:P, :])

                def u_vn_alloc(hh):
                    def go():
                        st[f"vn{hh}"] = vnp.tile(
                            [P, T // P, P], dt_mm, tag=f"vn{hh}", name="vn_h"
                        )

                    return go

                def u_vnx(hh):
                    # one batched XBAR transpose per head writes the whole
                    # PV lhsT tile -- dims, ones (denominator) column and
                    # zero padding: out[p, ct, r] = vtd[r, ct*128+p]
                    def go():
                        nc.sync.dma_start_transpose(
                            st[f"vn{hh}"][:],
                            vtd[hh * P : (hh + 1) * P, :],
                        )

                    return go

                units.append(u_vtd)
                for hh in range(2):
                    units.append(u_vn_alloc(hh))
                    units.append(u_vnx(hh))
                return st, units

            def pump(units, n):
                for _ in range(n):
                    if units:
                        units.pop(0)()

            def attention(pr, st, next_units, pump_rate, pump_start=0):
                """Attention for pair pr using st['qt'/'kt'/'vn*'], pumping
                next pair's units between chunk iterations.  Each quarter's
                unnormalized rows are evicted to bf16 (head 0 at partitions
                0:64, head 1 at 64:128) and its denominator row collected
                into a [1, 4NQ] f32 row.  At pair end the row bounces
                through DRAM into a [128, 16] tile so the reciprocal runs on
                all DVE lanes, then bounces back as a bf16 row.  The
                broadcast matmul + normalize multiply are returned as units
                pumped during the NEXT pair so the PE never waits on the
                reciprocal chain."""
                sums = sumsp.tile([1, 4 * NQ], F32, tag="sums", name="sums")
                aus = [
                    aup.tile([P, NQ], dt_mm, tag=f"au{qb}", name="au")
                    for qb in range(TQ // NQ)
                ]
                for hh in range(2):
                    h0 = hh * HD
                    den_r = HD if hh == 0 else 0  # denom row in po
                    for qb in range(TQ // NQ):
                        qs = slice(qb * NQ, (qb + 1) * NQ)
                        seg = hh * 2 + qb
                        po = ps_po.tile([P, NQ], F32, tag="po", name="po")
                        pending_pv = None
                        for ci in range(T // P // 2):
                            ps2 = ps_sc.tile([P, 2 * NQ], F32, tag="sc", name="ps2")
                            for k in range(2):
                                ch = 2 * ci + k
                                nc.tensor.matmul(
                                    ps2[:, k * NQ : (k + 1) * NQ],
                                    st["kt"][h0 : h0 + HD, ch * P : (ch + 1) * P],
                                    st["qt"][h0 : h0 + HD, qs],
                                    start=True,
                                    stop=True,
                                )
                            pexp = exp_p.tile(
                                [P, 2 * NQ], dt_mm, tag="ex", name="pexp"
                            )
                            nc.scalar.activation(
                                pexp[:],
                                ps2[:],
                                mybir.ActivationFunctionType.Exp,
                                scale=float(SCALE),
                            )
                            if seg * (T // P // 2) + ci >= pump_start:
                                pump(next_units, pump_rate)
                            # PV runs one iteration behind so exp has a full
                            # iteration of latency to hide
                            if pending_pv is not None:
                                pending_pv()
                            def make_pv(pexp=pexp, ci=ci):
                                def go():
                                    for k in range(2):
                                        ch = 2 * ci + k
                                        nc.tensor.matmul(
                                            po[:],
                                            st[f"vn{hh}"][:, ch, :],
                                            pexp[:, k * NQ : (k + 1) * NQ],
                                            start=(ch == 0),
                                            stop=(ch == T // P - 1),
                                        )
                                return go
                            pending_pv = make_pv()
                        pending_pv()
                        # evict unnormalized rows + denominator row
                        nc.scalar.copy(
                            aus[qb][h0 : h0 + HD, :], po[h0 : h0 + HD, :]
                        )
                        nc.vector.tensor_copy(
                            sums[:, seg * NQ : (seg + 1) * NQ],
                            po[den_r : den_r + 1, :],
                        )
                # reciprocal on 128 lanes via DRAM-bounce transpose
                nc.sync.dma_start(sumsd[pr : pr + 1, :], sums[:])
                t128 = sumsp.tile([P, 4 * NQ // P], F32, tag="t128", name="t128")
                nc.sync.dma_start(
                    t128[:],
                    sumsd[pr : pr + 1, :].rearrange("a (p j) -> (a p) j", p=P),
                )
                r128f = recp.tile([P, 4 * NQ // P], F32, tag="r128f", name="r128f")
                nc.vector.reciprocal(r128f[:], t128[:])
                r128b = recp.tile([P, 4 * NQ // P], dt_mm, tag="r128b", name="r128b")
                nc.scalar.copy(r128b[:], r128f[:])
                nc.sync.dma_start(
                    recd[pr : pr + 1, :].rearrange("a (p j) -> (a p) j", p=P),
                    r128b[:],
                )
                rrow = recp.tile([1, 4 * NQ], dt_mm, tag="rrow", name="rrow")
                nc.sync.dma_start(rrow[:], recd[pr : pr + 1, :])

                def make_norm(seg):
                    hh, qb = divmod(seg, 2)
                    h0 = hh * HD
                    qs = slice(qb * NQ, (qb + 1) * NQ)

                    def go():
                        pb = pbpool["pool"].tile([P, NQ], F32, tag="sc", name="pb")
                        nc.tensor.matmul(
                            pb[h0 : h0 + HD, :],
                            ones64[:],
                            rrow[0:1, seg * NQ : (seg + 1) * NQ],
                            start=True,
                            stop=True,
                        )
                        nc.vector.tensor_mul(
                            attn[pr][h0 : h0 + HD, qs],
                            aus[qb][h0 : h0 + HD, :],
                            pb[h0 : h0 + HD, :],
                        )

                    return go

                return [make_norm(s) for s in range(4)]

            def vtd_init():
                # dedicated pool: sharing vt's buffer would make pair-0's
                # V evictions wait for these DMAs
                zt = ztp.tile([P, T], dt_mm, tag="zt", name="zt")
                nc.vector.memset(zt[:], 0.0)
                onesrow = persist.tile([1, T], dt_mm, tag="onesrow", name="onesrow")
                nc.vector.memset(onesrow[:], 1.0)
                nc.sync.dma_start(vtd[HD : HD + 1, :], onesrow[:])
                nc.sync.dma_start(vtd[P : P + 1, :], onesrow[:])
                nc.sync.dma_start(vtd[HD + 1 : P, :], zt[0 : P - HD - 1, :])
                nc.sync.dma_start(vtd[P + 1 : P + HD, :], zt[0 : HD - 1, :])

            st, units = stage_units(0)
            # critical pair-0 loads (weights + x chunks) go to the sprayed
            # DMA queues first; vtd constants + cos/sin follow
            pump(units, 7)
            vtd_init()
            pump(units, len(units))
            norm_prev = []
            for pr in range(NPAIR):
                if pr + 1 < NPAIR:
                    nxt_st, nxt_units = stage_units(pr + 1)
                else:
                    nxt_st, nxt_units = None, []
                all_units = nxt_units + norm_prev
                pump_rate = (len(all_units) + 29) // 30 if all_units else 0
                # with only norm units left (pair 7), delay pumping until
                # the pair-6 reciprocal DRAM bounce has surely landed
                pump_start = 0 if nxt_units else 12
                norm_prev = attention(pr, st, all_units, pump_rate, pump_start)
                pump(all_units, len(all_units))
                st = nxt_st

        # ---- output projection; pair 7's normalization interleaves after
        # the first two blocks' ch0..6 matmuls so the PE never drains ----
        with contextlib.ExitStack() as ctx:
            wop = ctx.enter_context(tc.tile_pool(name="wop", bufs=1))
            outp = ctx.enter_context(tc.tile_pool(name="outp", bufs=3))
            ps_o = ctx.enter_context(
                tc.tile_pool(name="ps_o", bufs=4, space="PSUM")
            )
            pbpool["pool"] = ps_o
            bob_t = persist.tile([P, D], F32, tag="bob_t", name="bob_t")
            nc.sync.dma_start(bob_t[:], bob[:])
            wo_c = []
            for ch in range(NPAIR):
                wo_ch = wop.tile([P, D], dt_mm, tag=f"wo{ch}", name="wo_ch")
                nc.sync.dma_start(wo_ch[:], wot[ch * P : (ch + 1) * P, :])
                wo_c.append(wo_ch)

            def finish_tb(tb, pout):
                ts = slice(tb * P, (tb + 1) * P)
                for nh in range(2):
                    nc.tensor.matmul(
                        pout[nh][:],
                        attn[NPAIR - 1][:, ts],
                        wo_c[NPAIR - 1][:, nh * NQ : (nh + 1) * NQ],
                        start=False,
                        stop=True,
                    )
                osb = outp.tile([P, D], F32, tag="osb", name="osb")
                for nh in range(2):
                    nc.vector.tensor_add(
                        osb[:, nh * NQ : (nh + 1) * NQ],
                        pout[nh][:],
                        bob_t[:, nh * NQ : (nh + 1) * NQ],
                    )
                nc.sync.dma_start(out[ts, :], osb[:])

            open_q = []
            for tb in range(TQ // P):
                ts = slice(tb * P, (tb + 1) * P)
                pout = [
                    ps_o.tile([P, NQ], F32, tag="pout", name="pout")
                    for _ in range(2)
                ]
                for ch in range(NPAIR - 1):
                    for nh in range(2):
                        nc.tensor.matmul(
                            pout[nh][:],
                            attn[ch][:, ts],
                            wo_c[ch][:, nh * NQ : (nh + 1) * NQ],
                            start=(ch == 0),
                            stop=False,
                        )
                open_q.append((tb, pout))
                if tb == 1:
                    pump(norm_prev, len(norm_prev))
                if tb >= 1:
                    finish_tb(*open_q.pop(0))
            finish_tb(*open_q.pop(0))

        outer_cm.close()
        persist_cm.__exit__(None, None, None)

    _split_sync_waits(nc)
    return nc


# ---------------- host-side input prep ----------------
def _np_dt(dt_mm):
    return ml_dtypes.bfloat16 if dt_mm == mybir.dt.bfloat16 else np.float32


def _cs_tiles(frac_b):
    """cos/sin [32, T] f32 RoPE base patterns for one batch (frac_b: [T]).
    The device replicates to 128 rows (csa = 4x cos, csb = [sin; -sin] x2)."""
    i = np.arange(HALF, dtype=np.float64)
    freq = (ROPE_BASE ** (2.0 * i / HD)).astype(np.float32)  # [32]
    pos = frac_b.astype(np.float32) * np.float32(ROPE_SCALE)
    ang = pos[None, :] / freq[:, None]  # [32, T] f32
    a64 = ang.astype(np.float64)
    cos = np.cos(a64).astype(np.float32)
    sin = np.sin(a64).astype(np.float32)
    return np.ascontiguousarray(cos), np.ascontiguousarray(sin)


def _block_pt(a, nblk):
    """[D, N] -> [N//nblk, P, D//P, nblk]: out[nb, p, f, j] = a[f*P+p, nb*nblk+j]."""
    d, n = a.shape
    return np.ascontiguousarray(
        a.reshape(d // P, P, n // nblk, nblk).transpose(2, 1, 0, 3)
    )


def make_in_maps(x, frac, Wq, bq, Wk, bk, Wv, bv, Wo, bo, dt_mm=DT_MM):
    npdt = _np_dt(dt_mm)
    wqtb = _block_pt(np.ascontiguousarray(Wq.T).astype(npdt), P)
    wktb = _block_pt(np.ascontiguousarray(Wk.T).astype(npdt), P)
    wvtb = _block_pt(np.ascontiguousarray(Wv.T).astype(npdt), P)
    wot = np.ascontiguousarray(Wo.T).astype(npdt)
    bq_t = np.ascontiguousarray(bq.reshape(NPAIR, P).T).astype(np.float32)
    bk_t = np.ascontiguousarray(bk.reshape(NPAIR, P).T).astype(np.float32)
    bv_t = np.ascontiguousarray(bv.reshape(NPAIR, P).T).astype(np.float32)
    bob = np.ascontiguousarray(np.tile(bo[None, :], (P, 1))).astype(np.float32)
    in_maps = []
    for c in range(N_CORES):
        b, tqh = c // 2, c % 2
        xt = np.ascontiguousarray(x[b].T).astype(npdt)  # [D, T]
        xtq = np.ascontiguousarray(xt[:, tqh * TQ : (tqh + 1) * TQ])
        csa, csb = _cs_tiles(frac[b])
        in_maps.append(
            {
                "xtb": _block_pt(xt, NQ),
                "xtqb": _block_pt(xtq, NQ),
                "wqtb": wqtb,
                "wktb": wktb,
                "wvtb": wvtb,
                "wot": wot,
                "bq": bq_t,
                "bk": bk_t,
                "bv": bv_t,
                "bob": bob,
                "csaq": np.ascontiguousarray(
                    csa[:, tqh * TQ : (tqh + 1) * TQ]
                ).astype(npdt),
                "csbq": np.ascontiguousarray(
                    csb[:, tqh * TQ : (tqh + 1) * TQ]
                ).astype(npdt),
                "csak": csa.astype(npdt),
                "csbk": csb.astype(npdt),
            }
        )
    return in_maps


_NC_CACHE = {}


def _get_nc(dt_mm=DT_MM):
    key = str(dt_mm)
    if key not in _NC_CACHE:
        _NC_CACHE[key] = build_nc(dt_mm)
    return _NC_CACHE[key]


def kernel(x, frac, Wq, bq, Wk, bk, Wv, bv, Wo, bo):
    install_shims()
    from concourse.bass_utils import run_bass_kernel_spmd

    x = np.asarray(x, dtype=np.float32)
    frac = np.asarray(frac, dtype=np.float32)
    args = [np.asarray(a, dtype=np.float32) for a in (Wq, bq, Wk, bk, Wv, bv, Wo, bo)]
    in_maps = make_in_maps(x, frac, *args, dt_mm=DT_MM)
    nc = _get_nc(DT_MM)
    res = run_bass_kernel_spmd(nc, in_maps, list(range(N_CORES)))
    out = np.empty((B, T, D), dtype=np.float32)
    for c in range(N_CORES):
        b, tqh = c // 2, c % 2
        out[b, tqh * TQ : (tqh + 1) * TQ, :] = res.results[c]["out"]
    return out



# revision 74
# speedup vs baseline: 1.0062x; 1.0062x over previous
"""Multi-head attention with fraction-based RoPE ("stoich RoPE") on 8
Trainium2 NeuronCores.

Sharding: each core owns one (batch, query-half) pair — B=4 batches x 2
query halves = 8 shards.  Every core projects Q for its 1024 query rows
and K/V for the full 2048 keys of its batch (K/V projection is computed
on both cores sharing a batch; the 2x redundancy buys a kernel with no
collectives: the attention output rows owned by a core carry the full
head dimension, so the output projection and bias are entirely local).

Per-core device program (SPMD, identical on all 8 cores), pipelined so
pair pr+1's projections and pair pr-1's softmax normalization are pumped
into pair pr's attention instruction stream (the PE never waits on DMA,
rope, or the reciprocal chain):
  stage    per head-pair (8 x 128 dims): project Q^T/K^T/V^T from
           host-blocked x^T chunks (weights stationary, x moving, bias
           added at PSUM eviction), RoPE Q/K on DVE per 512-column chunk
           (the 32-row cross-quadrant swap is done by partition-shifted
           multiplies; cos/sin tiles are sent as 32 host rows and
           replicated on-chip), stage V^T dims into a DRAM buffer whose
           row layout equals the P@V' lhsT column layout (ones-denominator
           column and zero padding are pre-written constants), then one
           batched XBAR DMA transpose per head writes the whole [128, 16,
           128] PV lhsT tile.
  attn     per head: scores^T = K^T.T @ Q^T chunks -> exp on ACT
           (scale=1/8 folded in, no max subtraction: |scores/8| is O(1)
           for this operator's input distribution) -> P^T@V' accumulation
           (head 0 lands at PSUM partitions 0:64, head 1 at 64:128, each
           with its denominator row) -> unnormalized rows evicted to bf16.
           Denominator rows bounce through DRAM into a [128, 16] tile so
           the reciprocal runs on all DVE lanes, bounce back as a bf16
           row, and the K=1 broadcast matmul + normalize multiply are
           pumped during the NEXT pair.
  out      output projection: attn^T chunks stationary, Wo^T moving, +
           bias; the last pair's normalization interleaves after the
           first two row-blocks' ch0..6 matmuls (ch7 is deferred).

The host shards/formats inputs (transposes, DMA-contiguous blocking,
bias/cos-sin tiles) and concatenates the 8 output row-shards.
"""

import contextlib
import ctypes
import sys
import types

import numpy as np
import ml_dtypes

import concourse.bass as bass
import concourse.mybir as mybir
import concourse.tile as tile
from concourse import library_config
from concourse.masks import make_identity
from concourse.vector_clock import ScopedClock

# ---------------- problem constants (hardcoded per contract) ----------------
B, T, D = 4, 2048, 1024
H, HD = 16, 64  # heads, head dim
HALF = HD // 2
N_CORES = 8
TQ = T // 2  # query rows per core
P = 128
NQ = 512  # moving-dim tile for matmuls
NPAIR = D // P  # 8 head pairs per core
SCALE = 1.0 / np.sqrt(HD)  # folded into exp()
ROPE_SCALE = 1000.0
ROPE_BASE = 10000.0

F32 = mybir.dt.float32
DT_MM = mybir.dt.bfloat16  # dtype of matmul operands (bfloat16 | float32)

_SO_PATH = "/opt/axon/libaxon_pjrt.so"


# ---------------- axon/NTFF environment shims ----------------
def _ntff_profile_hook():
    try:
        lib = ctypes.CDLL(_SO_PATH)
    except OSError:
        return None
    if not hasattr(lib, "axon_start_nrt_profile"):
        return None
    lib.axon_start_nrt_profile.argtypes = [
        ctypes.POINTER(ctypes.c_int64),
        ctypes.c_size_t,
    ]
    lib.axon_start_nrt_profile.restype = ctypes.c_int64
    lib.axon_stop_nrt_profile.argtypes = [ctypes.c_char_p]
    lib.axon_stop_nrt_profile.restype = ctypes.c_int64

    @contextlib.contextmanager
    def _hook(output_dir, device_ids):
        import jax

        jax.devices()
        if device_ids:
            ids = (ctypes.c_int64 * len(device_ids))(*device_ids)
            rc = lib.axon_start_nrt_profile(ids, len(device_ids))
        else:
            rc = lib.axon_start_nrt_profile(None, 0)
        if rc != 0:
            raise RuntimeError(f"axon_start_nrt_profile rc={rc}")
        try:
            yield
        finally:
            n = lib.axon_stop_nrt_profile(str(output_dir).encode())
            if n < 0:
                raise RuntimeError(f"axon_stop_nrt_profile rc={n}")

    return _hook


def install_shims():
    if "antenv.axon_hooks" not in sys.modules:
        mod = types.ModuleType("antenv.axon_hooks")
        hook = _ntff_profile_hook()
        mod.get_axon_ntff_profile_hook = lambda: hook
        mod.set_axon_ntff_profile_hook = lambda h: None
        sys.modules["antenv.axon_hooks"] = mod
    import concourse.bass_utils as bass_utils

    bass_utils.upload_artifacts = lambda tmpdir: str(tmpdir)

    import os

    if os.environ.get("BASS_LDW_OPT") == "1" and not getattr(
        bass_utils, "_ldw_opt_patched", False
    ):
        orig_run = bass_utils.run_command

        def _run_ldw(argv, **kw):
            argv = [
                "--enable-ldw-opt=true" if a == "--enable-ldw-opt=false" else a
                for a in argv
            ]
            return orig_run(argv, **kw)

        bass_utils.run_command = _run_ldw
        bass_utils._ldw_opt_patched = True


class TileContextSplitDrain(tile.TileContext):
    """This walrus build encodes at most 2 sync waits per CTRL
    instruction; Tile's kernel-tail drain wants one wait per logical
    processor.  Split the waits across single-wait NOPs instead."""

    MAX_WAITS = 1

    def _drain_and_barrier(self, tick_clock, wait_clock):
        nc = self.nc
        carrier = nc.sync.nop(nofuse=True)
        wait_clock.add_sem_waits(
            carrier.ins, ScopedClock({None: tick_clock.global_clock})
        )
        waits = list(carrier.ins.sync_info.on_wait or [])
        if len(waits) > self.MAX_WAITS:
            carrier.ins.sync_info.on_wait[:] = waits[: self.MAX_WAITS]
            for i in range(self.MAX_WAITS, len(waits), self.MAX_WAITS):
                extra = nc.sync.nop(nofuse=True)
                extra.ins.sync_info = mybir.SyncInfo(
                    on_wait=list(waits[i : i + self.MAX_WAITS]), on_update=[]
                )
        nc.sync.drain()
        nc.all_engine_barrier()
        assert self.sems is not None
        popped = nc._tile_sem_poison_stack.pop()
        assert popped is self._sem_poison
        nc.clear_and_free_semaphores(list(self.sems.allocated().values()))
        nc.all_engine_barrier()


def _split_sync_waits(nc, max_waits=1):
    """This walrus build rejects instructions carrying more than a couple
    of sync waits (matmul takes 2, activation only 1).  Move excess waits
    onto same-engine NOPs inserted just before the instruction (AND
    semantics are preserved: the engine blocks on each carrier in program
    order)."""
    for f in nc.m.functions:
        for bb in f.blocks:
            out = []
            for inst in bb.instructions:
                mw = max_waits
                si = inst.sync_info
                waits = list(si.on_wait) if si and si.on_wait else []
                if len(waits) > mw:
                    for i in range(0, len(waits) - mw, mw):
                        nop = mybir.InstNoOp(
                            name=nc.get_next_instruction_name(), ins=[], outs=[]
                        )
                        nop.engine = inst.engine
                        nop.sync_info = mybir.SyncInfo(
                            on_wait=list(waits[i : i + mw]), on_update=[]
                        )
                        nc.register_instruction(nop, overwrite=True)
                        out.append(nop)
                    si.on_wait[:] = waits[len(waits) - mw :]
                out.append(inst)
            bb.instructions[:] = out


# ---------------- device program ----------------
def build_nc(dt_mm=DT_MM):
    nc = bass.Bass(
        "TRN2", target_bir_lowering=False, debug=False, num_devices=N_CORES
    )

    # x/weight layouts are host-blocked so each device DMA is contiguous
    # per partition (128 descriptors instead of 1024 per transfer):
    #   xtb[nb, p, f, t'] = x.T[f*128+p, nb*NQ+t']   (same for xtqb)
    #   wXb[pr, p, f, d'] = WX.T[f*128+p, pr*128+d']
    xtb = nc.dram_tensor("xtb", [T // NQ, P, NPAIR, NQ], dt_mm, kind="ExternalInput")
    xtqb = nc.dram_tensor(
        "xtqb", [TQ // NQ, P, NPAIR, NQ], dt_mm, kind="ExternalInput"
    )
    wqtb = nc.dram_tensor("wqtb", [NPAIR, P, NPAIR, P], dt_mm, kind="ExternalInput")
    wktb = nc.dram_tensor("wktb", [NPAIR, P, NPAIR, P], dt_mm, kind="ExternalInput")
    wvtb = nc.dram_tensor("wvtb", [NPAIR, P, NPAIR, P], dt_mm, kind="ExternalInput")
    wot = nc.dram_tensor("wot", [D, D], dt_mm, kind="ExternalInput")
    bq = nc.dram_tensor("bq", [P, NPAIR], F32, kind="ExternalInput")
    bk = nc.dram_tensor("bk", [P, NPAIR], F32, kind="ExternalInput")
    bv = nc.dram_tensor("bv", [P, NPAIR], F32, kind="ExternalInput")
    bob = nc.dram_tensor("bob", [P, D], F32, kind="ExternalInput")
    # cos/sin tiles are 4x row-replications of a [32, T] pattern; the host
    # sends 32 rows, gpsimd replicates on-chip (saves 1.1MB of startup DMA)
    csaq = nc.dram_tensor("csaq", [HALF, TQ], dt_mm, kind="ExternalInput")
    csbq = nc.dram_tensor("csbq", [HALF, TQ], dt_mm, kind="ExternalInput")
    csak = nc.dram_tensor("csak", [HALF, T], dt_mm, kind="ExternalInput")
    csbk = nc.dram_tensor("csbk", [HALF, T], dt_mm, kind="ExternalInput")
    out = nc.dram_tensor("out", [TQ, D], F32, kind="ExternalOutput")
    # DRAM bounce buffers: softmax denominators go out as a [1, 4NQ] row
    # and come back as [128, 16] so the reciprocal runs on 128 lanes.
    sumsd = nc.dram_tensor("sumsd", [NPAIR, 4 * NQ], F32, kind="Internal")
    recd = nc.dram_tensor("recd", [NPAIR, 4 * NQ], dt_mm, kind="Internal")
    # V staging for the XBAR transpose.  Row layout = the PV lhsT column
    # layout: rows 0:64 head-0 dims, row 64 ones, 65:128 zeros (-> vn_h0);
    # rows 128 ones, 129:192 zeros, 192:256 head-1 dims (-> vn_h1).  The
    # ones/zero rows are written once at startup; per pair only the dim
    # rows are overwritten (pair pr+1's write naturally waits for pair
    # pr's transposes, which complete early in pr's attention).
    vtd = nc.dram_tensor("vtd", [2 * P, T], dt_mm, kind="Internal")

    with TileContextSplitDrain(nc) as tc:
        persist_cm = tc.tile_pool(name="persist", bufs=1)
        persist = persist_cm.__enter__()

        def ptile(shape, dt, tag):
            return persist.tile(shape, dt, tag=tag, name=tag)

        # pools that outlive the attention scope (pair 7's normalization
        # units are pumped inside the output-projection scope)
        outer_cm = contextlib.ExitStack()
        aup = outer_cm.enter_context(tc.tile_pool(name="aup", bufs=2))
        recp = outer_cm.enter_context(tc.tile_pool(name="recp", bufs=2))
        pbpool = {}

        with contextlib.ExitStack() as ctx:
            # ---- persistent tiles ----
            csaq_t = ptile([P, TQ], dt_mm, "csaq_t")
            csbq_t = ptile([P, TQ], dt_mm, "csbq_t")
            csak_t = ptile([P, T], dt_mm, "csak_t")
            csbk_t = ptile([P, T], dt_mm, "csbk_t")
            bq_t = ptile([P, NPAIR], F32, "bq_t")
            bk_t = ptile([P, NPAIR], F32, "bk_t")
            bv_t = ptile([P, NPAIR], F32, "bv_t")
            attn = [ptile([P, TQ], dt_mm, f"attn{pr}") for pr in range(NPAIR)]
            ones64 = ptile([1, HD], dt_mm, "ones64")
            nc.vector.memset(ones64[:], 1.0)

            # biases and the 32-row cos/sin patterns are tiny and gate the
            # first PSUM evictions / ropes: load them before everything
            nc.sync.dma_start(bq_t[:], bq[:])
            nc.sync.dma_start(bk_t[:], bk[:])
            nc.sync.dma_start(bv_t[:], bv[:])
            nc.sync.dma_start(csaq_t[0:HALF, :], csaq[:])
            nc.sync.dma_start(csbq_t[0:HALF, :], csbq[:])
            nc.sync.dma_start(csak_t[0:HALF, :], csak[:])
            nc.sync.dma_start(csbk_t[0:HALF, :], csbk[:])
            # replicate to 128 rows on DVE (idle at startup; gpsimd is far
            # too slow): csa = 4x cos pattern; csb = [sin; -sin; sin; -sin]
            for cs_t in (csaq_t, csak_t):
                for r in (1, 2, 3):
                    nc.vector.tensor_copy(
                        cs_t[r * HALF : (r + 1) * HALF, :], cs_t[0:HALF, :]
                    )
            for cs_t in (csbq_t, csbk_t):
                nc.vector.tensor_copy(cs_t[2 * HALF : 3 * HALF, :], cs_t[0:HALF, :])
                for r in (1, 3):
                    nc.vector.tensor_scalar_mul(
                        cs_t[r * HALF : (r + 1) * HALF, :], cs_t[0:HALF, :], -1.0
                    )

            # ---- pools for the head-pair loop ----
            big = 2 if dt_mm != F32 else 1
            xp = ctx.enter_context(tc.tile_pool(name="xp", bufs=4))
            wp = ctx.enter_context(tc.tile_pool(name="wp", bufs=2))
            rawp = ctx.enter_context(tc.tile_pool(name="rawp", bufs=2))
            ropep = ctx.enter_context(tc.tile_pool(name="ropep", bufs=1))
            vtp = ctx.enter_context(tc.tile_pool(name="vtp", bufs=1))
            qkp = ctx.enter_context(tc.tile_pool(name="qkp", bufs=big))
            vnp = ctx.enter_context(tc.tile_pool(name="vnp", bufs=big))
            exp_p = ctx.enter_context(tc.tile_pool(name="exp_p", bufs=6))
            sumsp = ctx.enter_context(tc.tile_pool(name="sumsp", bufs=2))
            ztp = ctx.enter_context(tc.tile_pool(name="ztp", bufs=1))
            ps_proj = ctx.enter_context(
                tc.tile_pool(name="ps_proj", bufs=2, space="PSUM")
            )
            ps_sc = ctx.enter_context(
                tc.tile_pool(name="ps_sc", bufs=2, space="PSUM")
            )
            ps_po = ctx.enter_context(
                tc.tile_pool(name="ps_po", bufs=2, space="PSUM")
            )
            pbpool["pool"] = ps_sc

            def rope_chunk(raw, csa_t, csb_t, out_tile, c0, c1):
                # one [P, c1-c0] chunk: out = raw*csa + swap32(raw*csb),
                # the swap done by partition-shifted multiplies
                n = c1 - c0
                cs = slice(c0, c1)
                m1 = ropep.tile([P, NQ], dt_mm, tag="m1", name="m1")
                m2s = ropep.tile([P, NQ], dt_mm, tag="m2s", name="m2s")
                nc.vector.tensor_mul(m1[:, :n], raw[:, cs], csa_t[:, cs])
                for blk in range(2):
                    b0 = blk * 64
                    nc.vector.tensor_mul(
                        m2s[b0 : b0 + 32, :n],
                        raw[b0 + 32 : b0 + 64, cs],
                        csb_t[b0 + 32 : b0 + 64, cs],
                    )
                    nc.vector.tensor_mul(
                        m2s[b0 + 32 : b0 + 64, :n],
                        raw[b0 : b0 + 32, cs],
                        csb_t[b0 : b0 + 32, cs],
                    )
                nc.vector.tensor_add(out_tile[:, cs], m1[:, :n], m2s[:, :n])

            def stage_units(pr):
                """Emission units for pair pr's projections + RoPE + V
                transpose.  Each unit emits a small instruction group; the
                attention loop of the previous pair pumps these so the PE
                stays dense while ACT works on exp."""
                d0 = pr * P
                st = {}
                units = []

                def u_wdma():
                    st["wq"] = wp.tile([P, NPAIR, P], dt_mm, tag="wq", name="wq_c")
                    st["wk"] = wp.tile([P, NPAIR, P], dt_mm, tag="wk", name="wk_c")
                    st["wv"] = wp.tile([P, NPAIR, P], dt_mm, tag="wv", name="wv_c")
                    # K first: the projection matmul stream starts with wk
                    nc.sync.dma_start(st["wk"][:], wktb[pr])
                    st["qraw"] = rawp.tile([P, TQ], dt_mm, tag="qraw", name="q_raw")
                    st["kraw"] = rawp.tile([P, T], dt_mm, tag="kraw", name="k_raw")
                    st["vt"] = vtp.tile([P, T], dt_mm, tag="vt", name="v_t")

                def u_wdma2():
                    nc.sync.dma_start(st["wv"][:], wvtb[pr])
                    nc.sync.dma_start(st["wq"][:], wqtb[pr])

                units.append(u_wdma)

                def u_xdma(key, nb, src):
                    def go():
                        xc = xp.tile([P, NPAIR, NQ], dt_mm, tag="xc", name="xc")
                        nc.sync.dma_start(xc[:], src[nb])
                        st[key] = xc

                    return go

                def u_mm(w_key, x_key, f, start, stop):
                    def go():
                        if start:
                            st["ps"] = ps_proj.tile([P, NQ], F32, tag="ps", name="ps")
                        nc.tensor.matmul(
                            st["ps"][:],
                            st[w_key][:, f, :],
                            st[x_key][:, f, :],
                            start=start,
                            stop=stop,
                        )

                    return go

                def u_evict(b_t, dst_key, dslice):
                    def go():
                        nc.scalar.activation(
                            st[dst_key][:, dslice],
                            st["ps"][:],
                            mybir.ActivationFunctionType.Identity,
                            bias=b_t[:, pr : pr + 1],
                        )

                    return go

                # all DMAs first: deep prefetch so pumped matmuls never
                # wait on HBM
                units.append(u_xdma("x0", 0, xtb))
                units.append(u_wdma2)
                for nb in range(1, T // NQ):
                    units.append(u_xdma("x%d" % nb, nb, xtb))
                for nb in range(TQ // NQ):
                    units.append(u_xdma("q%d" % nb, nb, xtqb))
                def u_rope_k(nb):
                    def go():
                        if nb == 0:
                            st["kt"] = qkp.tile([P, T], dt_mm, tag="kt", name="kt")
                        rope_chunk(
                            st["kraw"], csak_t, csbk_t, st["kt"],
                            nb * NQ, (nb + 1) * NQ,
                        )

                    return go

                def u_rope_q(nb):
                    def go():
                        if nb == 0:
                            st["qt"] = qkp.tile([P, TQ], dt_mm, tag="qt", name="qt")
                        rope_chunk(
                            st["qraw"], csaq_t, csbq_t, st["qt"],
                            nb * NQ, (nb + 1) * NQ,
                        )

                    return go

                unit_pos_v1 = None
                for nb in range(T // NQ):
                    for w_key, b_t, dst_key in (("wk", bk_t, "kraw"), ("wv", bv_t, "vt")):
                        for f in range(NPAIR):
                            units.append(
                                u_mm(w_key, "x%d" % nb, f, f == 0, f == NPAIR - 1)
                            )
                        units.append(
                            u_evict(b_t, dst_key, slice(nb * NQ, (nb + 1) * NQ))
                        )
                    units.append(u_rope_k(nb))
                    if nb == 1:
                        unit_pos_v1 = len(units)
                for nb in range(TQ // NQ):
                    for f in range(NPAIR):
                        units.append(u_mm("wq", "q%d" % nb, f, f == 0, f == NPAIR - 1))
                    units.append(
                        u_evict(bq_t, "qraw", slice(nb * NQ, (nb + 1) * NQ))
                    )
                    units.append(u_rope_q(nb))

                def u_vtd(g):
                    # stage one key-half of V^T dims into the DRAM transpose
                    # buffer; the constant ones/zero rows are already there
                    gs = slice(g * TQ, (g + 1) * TQ)

                    def go():
                        nc.sync.dma_start(vtd[0:HD, gs], st["vt"][0:HD, gs])
                        nc.sync.dma_start(
                            vtd[3 * HD : 4 * HD, gs], st["vt"][HD:P, gs]
                        )

                    return go

                def u_vn_alloc(hh):
                    def go():
                        st[f"vn{hh}"] = vnp.tile(
                            [P, T // P, P], dt_mm, tag=f"vn{hh}", name="vn_h"
                        )

                    return go

                def u_vnx(hh, g):
                    # one batched XBAR transpose per (head, key-half) writes
                    # that half's PV lhsT chunks -- dims, ones (denominator)
                    # column and zero padding: out[p, ct, r] = in[r, ct*128+p]
                    def go():
                        nc.sync.dma_start_transpose(
                            st[f"vn{hh}"][:, g * (TQ // P) : (g + 1) * (TQ // P), :],
                            vtd[hh * P : (hh + 1) * P, g * TQ : (g + 1) * TQ],
                        )

                    return go

                # first key-half of vn as soon as V evictions nb0/nb1 land,
                # so the first PV matmuls never wait on the whole V
                first_half = [
                    u_vtd(0), u_vn_alloc(0), u_vnx(0, 0), u_vn_alloc(1), u_vnx(1, 0),
                ]
                units[unit_pos_v1:unit_pos_v1] = first_half
                units.append(u_vtd(1))
                units.append(u_vnx(0, 1))
                units.append(u_vnx(1, 1))
                return st, units

            def pump(units, n):
                for _ in range(n):
                    if units:
                        units.pop(0)()

            def attention(pr, st, next_units, pump_rate, pump_start=0):
                """Attention for pair pr using st['qt'/'kt'/'vn*'], pumping
                next pair's units between chunk iterations.  Each quarter's
                unnormalized rows are evicted to bf16 (head 0 at partitions
                0:64, head 1 at 64:128) and its denominator row collected
                into a [1, 4NQ] f32 row.  At pair end the row bounces
                through DRAM into a [128, 16] tile so the reciprocal runs on
                all DVE lanes, then bounces back as a bf16 row.  The
                broadcast matmul + normalize multiply are returned as units
                pumped during the NEXT pair so the PE never waits on the
                reciprocal chain."""
                sums = sumsp.tile([1, 4 * NQ], F32, tag="sums", name="sums")
                aus = [
                    aup.tile([P, NQ], dt_mm, tag=f"au{qb}", name="au")
                    for qb in range(TQ // NQ)
                ]
                for hh in range(2):
                    h0 = hh * HD
                    den_r = HD if hh == 0 else 0  # denom row in po
                    for qb in range(TQ // NQ):
                        qs = slice(qb * NQ, (qb + 1) * NQ)
                        seg = hh * 2 + qb
                        po = ps_po.tile([P, NQ], F32, tag="po", name="po")
                        pending_pv = None
                        for ci in range(T // P // 2):
                            ps2 = ps_sc.tile([P, 2 * NQ], F32, tag="sc", name="ps2")
                            for k in range(2):
                                ch = 2 * ci + k
                                nc.tensor.matmul(
                                    ps2[:, k * NQ : (k + 1) * NQ],
                                    st["kt"][h0 : h0 + HD, ch * P : (ch + 1) * P],
                                    st["qt"][h0 : h0 + HD, qs],
                                    start=True,
                                    stop=True,
                                )
                            pexp = exp_p.tile(
                                [P, 2 * NQ], dt_mm, tag="ex", name="pexp"
                            )
                            nc.scalar.activation(
                                pexp[:],
                                ps2[:],
                                mybir.ActivationFunctionType.Exp,
                                scale=float(SCALE),
                            )
                            if seg * (T // P // 2) + ci >= pump_start:
                                pump(next_units, pump_rate)
                            # PV runs one iteration behind so exp has a full
                            # iteration of latency to hide
                            if pending_pv is not None:
                                pending_pv()
                            def make_pv(pexp=pexp, ci=ci):
                                def go():
                                    for k in range(2):
                                        ch = 2 * ci + k
                                        nc.tensor.matmul(
                                            po[:],
                                            st[f"vn{hh}"][:, ch, :],
                                            pexp[:, k * NQ : (k + 1) * NQ],
                                            start=(ch == 0),
                                            stop=(ch == T // P - 1),
                                        )
                                return go
                            pending_pv = make_pv()
                        pending_pv()
                        # evict unnormalized rows + denominator row
                        nc.scalar.copy(
                            aus[qb][h0 : h0 + HD, :], po[h0 : h0 + HD, :]
                        )
                        nc.vector.tensor_copy(
                            sums[:, seg * NQ : (seg + 1) * NQ],
                            po[den_r : den_r + 1, :],
                        )
                # reciprocal on 128 lanes via DRAM-bounce transpose
                nc.sync.dma_start(sumsd[pr : pr + 1, :], sums[:])
                t128 = sumsp.tile([P, 4 * NQ // P], F32, tag="t128", name="t128")
                nc.sync.dma_start(
                    t128[:],
                    sumsd[pr : pr + 1, :].rearrange("a (p j) -> (a p) j", p=P),
                )
                r128f = recp.tile([P, 4 * NQ // P], F32, tag="r128f", name="r128f")
                nc.vector.reciprocal(r128f[:], t128[:])
                r128b = recp.tile([P, 4 * NQ // P], dt_mm, tag="r128b", name="r128b")
                nc.scalar.copy(r128b[:], r128f[:])
                nc.sync.dma_start(
                    recd[pr : pr + 1, :].rearrange("a (p j) -> (a p) j", p=P),
                    r128b[:],
                )
                rrow = recp.tile([1, 4 * NQ], dt_mm, tag="rrow", name="rrow")
                nc.sync.dma_start(rrow[:], recd[pr : pr + 1, :])

                def make_norm(seg):
                    hh, qb = divmod(seg, 2)
                    h0 = hh * HD
                    qs = slice(qb * NQ, (qb + 1) * NQ)

                    def go():
                        pb = pbpool["pool"].tile([P, NQ], F32, tag="sc", name="pb")
                        nc.tensor.matmul(
                            pb[h0 : h0 + HD, :],
                            ones64[:],
                            rrow[0:1, seg * NQ : (seg + 1) * NQ],
                            start=True,
                            stop=True,
                        )
                        nc.vector.tensor_mul(
                            attn[pr][h0 : h0 + HD, qs],
                            aus[qb][h0 : h0 + HD, :],
                            pb[h0 : h0 + HD, :],
                        )

                    return go

                return [make_norm(s) for s in range(4)]

            def vtd_init():
                # dedicated pool: sharing vt's buffer would make pair-0's
                # V evictions wait for these DMAs
                zt = ztp.tile([P, T], dt_mm, tag="zt", name="zt")
                nc.vector.memset(zt[:], 0.0)
                onesrow = persist.tile([1, T], dt_mm, tag="onesrow", name="onesrow")
                nc.vector.memset(onesrow[:], 1.0)
                nc.sync.dma_start(vtd[HD : HD + 1, :], onesrow[:])
                nc.sync.dma_start(vtd[P : P + 1, :], onesrow[:])
                nc.sync.dma_start(vtd[HD + 1 : P, :], zt[0 : P - HD - 1, :])
                nc.sync.dma_start(vtd[P + 1 : P + HD, :], zt[0 : HD - 1, :])

            st, units = stage_units(0)
            # critical pair-0 loads (weights + x chunks) go to the sprayed
            # DMA queues first; vtd constants + cos/sin follow
            pump(units, 7)
            vtd_init()
            pump(units, len(units))
            norm_prev = []
            for pr in range(NPAIR):
                if pr + 1 < NPAIR:
                    nxt_st, nxt_units = stage_units(pr + 1)
                else:
                    nxt_st, nxt_units = None, []
                all_units = nxt_units + norm_prev
                pump_rate = (len(all_units) + 29) // 30 if all_units else 0
                # with only norm units left (pair 7), delay pumping until
                # the pair-6 reciprocal DRAM bounce has surely landed
                pump_start = 0 if nxt_units else 12
                norm_prev = attention(pr, st, all_units, pump_rate, pump_start)
                pump(all_units, len(all_units))
                st = nxt_st

        # ---- output projection; pair 7's normalization interleaves after
        # the first two blocks' ch0..6 matmuls so the PE never drains ----
        with contextlib.ExitStack() as ctx:
            wop = ctx.enter_context(tc.tile_pool(name="wop", bufs=1))
            outp = ctx.enter_context(tc.tile_pool(name="outp", bufs=3))
            # pout gets 6 banks (three row-blocks in flight), pb the other 2
            ps_o = ctx.enter_context(
                tc.tile_pool(name="ps_o", bufs=2, space="PSUM")
            )
            pbpool["pool"] = ps_o
            bob_t = persist.tile([P, D], F32, tag="bob_t", name="bob_t")
            nc.sync.dma_start(bob_t[:], bob[:])
            wo_c = []
            for ch in range(NPAIR):
                wo_ch = wop.tile([P, D], dt_mm, tag=f"wo{ch}", name="wo_ch")
                nc.sync.dma_start(wo_ch[:], wot[ch * P : (ch + 1) * P, :])
                wo_c.append(wo_ch)

            def finish_tb(tb, pout):
                ts = slice(tb * P, (tb + 1) * P)
                for nh in range(2):
                    nc.tensor.matmul(
                        pout[nh][:],
                        attn[NPAIR - 1][:, ts],
                        wo_c[NPAIR - 1][:, nh * NQ : (nh + 1) * NQ],
                        start=False,
                        stop=True,
                    )
                osb = outp.tile([P, D], F32, tag="osb", name="osb")
                for nh in range(2):
                    nc.vector.tensor_add(
                        osb[:, nh * NQ : (nh + 1) * NQ],
                        pout[nh][:],
                        bob_t[:, nh * NQ : (nh + 1) * NQ],
                    )
                nc.sync.dma_start(out[ts, :], osb[:])

            open_q = []
            for tb in range(TQ // P):
                ts = slice(tb * P, (tb + 1) * P)
                pout = [
                    ps_o.tile([P, NQ], F32, tag="pout", name="pout", bufs=6)
                    for _ in range(2)
                ]
                for ch in range(NPAIR - 1):
                    for nh in range(2):
                        nc.tensor.matmul(
                            pout[nh][:],
                            attn[ch][:, ts],
                            wo_c[ch][:, nh * NQ : (nh + 1) * NQ],
                            start=(ch == 0),
                            stop=False,
                        )
                open_q.append((tb, pout))
                # three blocks of ch0..6 matmuls (~9us) cover the pair-7
                # reciprocal DRAM bounce before its broadcast matmuls
                if tb == 2:
                    pump(norm_prev, len(norm_prev))
                if tb >= 2:
                    finish_tb(*open_q.pop(0))
            while open_q:
                finish_tb(*open_q.pop(0))

        outer_cm.close()
        persist_cm.__exit__(None, None, None)

    _split_sync_waits(nc)
    return nc


# ---------------- host-side input prep ----------------
def _np_dt(dt_mm):
    return ml_dtypes.bfloat16 if dt_mm == mybir.dt.bfloat16 else np.float32


def _cs_tiles(frac_b):
    """cos/sin [32, T] f32 RoPE base patterns for one batch (frac_b: [T]).
    The device replicates to 128 rows (csa = 4x cos, csb = [sin; -sin] x2)."""
    i = np.arange(HALF, dtype=np.float64)
    freq = (ROPE_BASE ** (2.0 * i / HD)).astype(np.float32)  # [32]
    pos = frac_b.astype(np.float32) * np.float32(ROPE_SCALE)
    ang = pos[None, :] / freq[:, None]  # [32, T] f32
    a64 = ang.astype(np.float64)
    cos = np.cos(a64).astype(np.float32)
    sin = np.sin(a64).astype(np.float32)
    return np.ascontiguousarray(cos), np.ascontiguousarray(sin)


def _block_pt(a, nblk):
    """[D, N] -> [N//nblk, P, D//P, nblk]: out[nb, p, f, j] = a[f*P+p, nb*nblk+j]."""
    d, n = a.shape
    return np.ascontiguousarray(
        a.reshape(d // P, P, n // nblk, nblk).transpose(2, 1, 0, 3)
    )


def make_in_maps(x, frac, Wq, bq, Wk, bk, Wv, bv, Wo, bo, dt_mm=DT_MM):
    npdt = _np_dt(dt_mm)
    wqtb = _block_pt(np.ascontiguousarray(Wq.T).astype(npdt), P)
    wktb = _block_pt(np.ascontiguousarray(Wk.T).astype(npdt), P)
    wvtb = _block_pt(np.ascontiguousarray(Wv.T).astype(npdt), P)
    wot = np.ascontiguousarray(Wo.T).astype(npdt)
    bq_t = np.ascontiguousarray(bq.reshape(NPAIR, P).T).astype(np.float32)
    bk_t = np.ascontiguousarray(bk.reshape(NPAIR, P).T).astype(np.float32)
    bv_t = np.ascontiguousarray(bv.reshape(NPAIR, P).T).astype(np.float32)
    bob = np.ascontiguousarray(np.tile(bo[None, :], (P, 1))).astype(np.float32)
    in_maps = []
    for c in range(N_CORES):
        b, tqh = c // 2, c % 2
        xt = np.ascontiguousarray(x[b].T).astype(npdt)  # [D, T]
        xtq = np.ascontiguousarray(xt[:, tqh * TQ : (tqh + 1) * TQ])
        csa, csb = _cs_tiles(frac[b])
        in_maps.append(
            {
                "xtb": _block_pt(xt, NQ),
                "xtqb": _block_pt(xtq, NQ),
                "wqtb": wqtb,
                "wktb": wktb,
                "wvtb": wvtb,
                "wot": wot,
                "bq": bq_t,
                "bk": bk_t,
                "bv": bv_t,
                "bob": bob,
                "csaq": np.ascontiguousarray(
                    csa[:, tqh * TQ : (tqh + 1) * TQ]
                ).astype(npdt),
                "csbq": np.ascontiguousarray(
                    csb[:, tqh * TQ : (tqh + 1) * TQ]
                ).astype(npdt),
                "csak": csa.astype(npdt),
                "csbk": csb.astype(npdt),
            }
        )
    return in_maps


_NC_CACHE = {}


def _get_nc(dt_mm=DT_MM):
    key = str(dt_mm)
    if key not in _NC_CACHE:
        _NC_CACHE[key] = build_nc(dt_mm)
    return _NC_CACHE[key]


def kernel(x, frac, Wq, bq, Wk, bk, Wv, bv, Wo, bo):
    install_shims()
    from concourse.bass_utils import run_bass_kernel_spmd

    x = np.asarray(x, dtype=np.float32)
    frac = np.asarray(frac, dtype=np.float32)
    args = [np.asarray(a, dtype=np.float32) for a in (Wq, bq, Wk, bk, Wv, bv, Wo, bo)]
    in_maps = make_in_maps(x, frac, *args, dt_mm=DT_MM)
    nc = _get_nc(DT_MM)
    res = run_bass_kernel_spmd(nc, in_maps, list(range(N_CORES)))
    out = np.empty((B, T, D), dtype=np.float32)
    for c in range(N_CORES):
        b, tqh = c // 2, c % 2
        out[b, tqh * TQ : (tqh + 1) * TQ, :] = res.results[c]["out"]
    return out

